# revision 6
# baseline (speedup 1.0000x reference)
"""Trainium2 Bass kernel for nn_FCVI_Net_78864189489850.

Computation (reference):
  L = lower-tri scatter of cov_vector (exp on diag)          [769, 769]
  samples = mean + L @ z                                      [769, S, B]
  W0 = samples[0:256], b0 = samples[256:512],
  W1 = samples[512:768], b1 = samples[768]
  h = relu(x * W0 + b0);  out = sum_o h * W1 + b1             [S, B]

Strategy (8 NeuronCores, batch-sharded, no cross-device comms):
  - Transposed orientation sT[c, i] = sum_k z[k,c] * LT[k,i], 32 c-tiles
    of 128 columns per core, f16 matmuls with fine triangular tiling.
  - Host pre-scales z k-tiles 0,1 by x (zx = x_c * z[k,c]); the pU psum
    group then accumulates u = x*sT0 + sT1 directly in one [128,256]
    region (zx tiles stream LT cols [0:256), plain z tiles stream LT
    cols [256:512) into the same psum columns).  pB accumulates W1/b1.
  - Epilogue: ACT copies pB->SBUF bf16 (c2); DVE does u2 = pU + apar
    and g = relu(u2)*v via stt(max,mult) with free-dim accumulator;
    gpsimd does v = c2 + mean2 and the b1 column add.  All means are
    folded host-side into apar/mean2/z8p.
  - DMA: z layouts are contiguous per partition; two HWDGE queues
    (sync: z chunks, scalar: lt/zx/consts) with a small first chunk so
    matmuls start early; no SWDGE.
"""
import os
import numpy as np

P = 769
S = 16
B = 2048
NCORES = 8
BC = B // NCORES          # 256 batch per core
NCOL = S * BC             # 4096 columns per core
NCT = NCOL // 128         # 32 c-tiles per core
KT = 6                    # k-tiles (rows 0..767; row 768 handled on host)
LT_LO = [0, 128, 256, 384, 512, 640]
LT_W = [P - lo for lo in LT_LO]                 # 769,641,513,385,257,129
LT_OFF = [sum(LT_W[:t]) for t in range(KT)]
LT_TOT = sum(LT_W)                              # 2694
LT_SPLIT = LT_OFF[2]                            # t0,t1 | t2..t5
ZCH = [(0, 2), (2, 6), (6, 14), (14, 22), (22, 32)]   # z chunks (c-tiles)
XCH = [(0, 4), (4, 18), (18, 32)]                     # zx chunks

_cache = {}


def _build_program():
    import concourse.bacc as bacc
    import concourse.tile as tile
    from concourse import mybir

    f16 = mybir.dt.float16
    bf16 = mybir.dt.bfloat16
    f32 = mybir.dt.float32

    nc = bacc.Bacc("TRN2", target_bir_lowering=False, debug=False)

    zr_d = nc.dram_tensor("zr", [128, NCT * KT * 128], f16, kind="ExternalInput")
    zx_d = nc.dram_tensor("zx", [128, NCT * 2 * 128], f16, kind="ExternalInput")
    ltr_d = nc.dram_tensor("ltr", [128, LT_TOT], f16, kind="ExternalInput")
    cb_d = nc.dram_tensor("cb", [128, 800], bf16, kind="ExternalInput")
    out_d = nc.dram_tensor("out", [128, NCT], f32, kind="ExternalOutput")

    with tile.TileContext(nc) as tc:
        with (
            tc.tile_pool(name="zpool", bufs=1) as zpool,
            tc.tile_pool(name="ltpool", bufs=1) as ltpool,
            tc.tile_pool(name="cpool", bufs=1) as cpool,
            tc.tile_pool(name="work", bufs=4) as work,
            tc.tile_pool(name="pu", bufs=3, space="PSUM") as pu_pool,
            tc.tile_pool(name="pb", bufs=3, space="PSUM") as pb_pool,
        ):
            # --- DMAs: two HWDGE queues, critical tiles first ---
            ltr = ltpool.tile([128, LT_TOT], f16, tag="ltr")
            nc.scalar.dma_start(out=ltr[:, 0:LT_SPLIT],
                                in_=ltr_d.ap()[:, 0:LT_SPLIT])

            zxt = []
            for ci, (m0, m1) in enumerate(XCH):
                t = zpool.tile([128, (m1 - m0) * 2 * 128], f16, tag=f"zx{ci}")
                zxt.append(t)
            zrt = []
            for ci, (m0, m1) in enumerate(ZCH):
                t = zpool.tile([128, (m1 - m0) * KT * 128], f16, tag=f"zr{ci}")
                zrt.append(t)

            nc.sync.dma_start(out=zrt[0][:],
                              in_=zr_d.ap()[:, ZCH[0][0] * 768:ZCH[0][1] * 768])
            nc.scalar.dma_start(out=zxt[0][:],
                                in_=zx_d.ap()[:, XCH[0][0] * 256:XCH[0][1] * 256])
            nc.scalar.dma_start(out=ltr[:, LT_SPLIT:LT_TOT],
                                in_=ltr_d.ap()[:, LT_SPLIT:LT_TOT])
            cb = cpool.tile([128, 800], bf16, tag="cb")
            nc.scalar.dma_start(out=cb[:], in_=cb_d.ap()[:, :])
            for ci in range(1, len(ZCH)):
                m0, m1 = ZCH[ci]
                nc.sync.dma_start(out=zrt[ci][:],
                                  in_=zr_d.ap()[:, m0 * 768:m1 * 768])
            for ci in range(1, len(XCH)):
                m0, m1 = XCH[ci]
                nc.scalar.dma_start(out=zxt[ci][:],
                                    in_=zx_d.ap()[:, m0 * 256:m1 * 256])

            stag = cpool.tile([128, NCT], f32, tag="stag")
            s3st = cpool.tile([128, NCT], f32, tag="s3st")
            st2 = cpool.tile([128, NCT], f32, tag="st2")

            zchunk_of = {}
            for ci, (m0, m1) in enumerate(ZCH):
                for m in range(m0, m1):
                    zchunk_of[m] = (ci, m0)
            xchunk_of = {}
            for ci, (m0, m1) in enumerate(XCH):
                for m in range(m0, m1):
                    xchunk_of[m] = (ci, m0)

            for m in range(NCT):
                zci, zm0 = zchunk_of[m]
                xci, xm0 = xchunk_of[m]
                zt = zrt[zci]
                xt = zxt[xci]

                def zl(t):
                    o = ((m - zm0) * KT + t) * 128
                    return zt[:, o:o + 128]

                def xl(t):
                    o = ((m - xm0) * 2 + t) * 128
                    return xt[:, o:o + 128]

                pU = pu_pool.tile([128, 256], f32, tag="pU")
                pB = pb_pool.tile([128, 257], f32, tag="pB")
                # pU = x*sT0 + sT1: zx tiles stream LT[:, 0:256),
                # plain z tiles stream LT[:, 256:512) into the same cols.
                nc.tensor.matmul(pU[:, 0:256], xl(0),
                                 ltr[:, LT_OFF[0]:LT_OFF[0] + 256],
                                 start=True, stop=False)
                nc.tensor.matmul(pU[:, 128:256], xl(1),
                                 ltr[:, LT_OFF[1]:LT_OFF[1] + 128],
                                 start=False, stop=False)
                nc.tensor.matmul(pU[:, 0:256], zl(0),
                                 ltr[:, LT_OFF[0] + 256:LT_OFF[0] + 512],
                                 start=False, stop=False)
                nc.tensor.matmul(pU[:, 0:256], zl(1),
                                 ltr[:, LT_OFF[1] + 128:LT_OFF[1] + 384],
                                 start=False, stop=False)
                nc.tensor.matmul(pU[:, 0:256], zl(2),
                                 ltr[:, LT_OFF[2]:LT_OFF[2] + 256],
                                 start=False, stop=False)
                nc.tensor.matmul(pU[:, 128:256], zl(3),
                                 ltr[:, LT_OFF[3]:LT_OFF[3] + 128],
                                 start=False, stop=True)
                # pB: W1/b1 region, i in [512,769), k-tiles 0..5
                for t in range(KT):
                    lo = max(LT_LO[t], 512)
                    nc.tensor.matmul(pB[:, lo - 512:257], zl(t),
                                     ltr[:, LT_OFF[t] + lo - LT_LO[t]:
                                          LT_OFF[t] + LT_W[t]],
                                     start=(t == 0), stop=(t == 5))

                par = m % 2
                # c2 = bf16 copy of pB (ACT engine)
                c2 = work.tile([128, 257], bf16, tag="c2")
                nc.scalar.activation(c2[:], pB[:, 0:257],
                                     mybir.ActivationFunctionType.Copy)
                # u2 = pU + (x*mean0 + mean1)
                u2 = work.tile([128, 256], bf16, tag="u2")
                nc.vector.tensor_add(u2[:], pU[:, 0:256],
                                     cb[:, par * 256:par * 256 + 256])
                # v = sT2 + mean2   (gpsimd)
                v = work.tile([128, 256], bf16, tag="v")
                nc.gpsimd.tensor_add(v[:], c2[:, 0:256], cb[:, 512:768])
                # g = relu(u2) * v ; stag[:, m] = sum_o g
                g = work.tile([128, 256], bf16, tag="g")
                nc.vector.scalar_tensor_tensor(
                    out=g[:], in0=u2[:], scalar=0.0, in1=v[:],
                    op0=mybir.AluOpType.max, op1=mybir.AluOpType.mult,
                    accum_out=stag[:, m:m + 1])
                # b1 column (gpsimd)
                nc.gpsimd.tensor_add(s3st[:, m:m + 1], c2[:, 256:257],
                                     cb[:, 768 + m:769 + m])

                if m in (NCT // 2 - 1, NCT - 1):
                    h_ = 0 if m == NCT // 2 - 1 else 1
                    sl = slice(h_ * (NCT // 2), (h_ + 1) * (NCT // 2))
                    nc.vector.tensor_add(st2[:, sl], stag[:, sl], s3st[:, sl])
                    nc.sync.dma_start(out=out_d.ap()[:, sl], in_=st2[:, sl])

    nc.compile()
    return nc


def _prep_inputs(x, mean, cov_vector, z):
    import ml_dtypes
    bf16 = ml_dtypes.bfloat16

    L = np.zeros((P, P), dtype=np.float32)
    L[np.tril_indices(P)] = cov_vector
    d = np.diag(L).copy()
    L[np.diag_indices(P)] = np.exp(d)

    LT = np.ascontiguousarray(L.T)               # LT[k, i] = L[i, k]
    ltr = np.concatenate(
        [LT[128 * t:128 * (t + 1), LT_LO[t]:P] for t in range(KT)],
        axis=1).astype(np.float16)               # [128, 2694]

    z2 = z.reshape(P, S, B)
    in_maps = []
    for c in range(NCORES):
        zs = z2[:, :, c * BC:(c + 1) * BC].reshape(P, NCOL)
        za = zs[:768].astype(np.float32)
        zr = np.ascontiguousarray(
            za.astype(np.float16).reshape(KT, 128, NCT, 128)
            .transpose(1, 2, 0, 3)).reshape(128, NCT * KT * 128)

        xs = x[c * BC:(c + 1) * BC].astype(np.float32)
        xcol = np.tile(xs, S)                     # x per column j = xs[j%256]
        zx = (za[:256] * xcol[None, :]).astype(np.float16)
        zxr = np.ascontiguousarray(
            zx.reshape(2, 128, NCT, 128).transpose(1, 2, 0, 3)
        ).reshape(128, NCT * 2 * 128)

        z8 = zs[768].astype(np.float32)          # [4096]

        cbv = np.empty((128, 800), dtype=np.float32)
        cbv[:, 0:256] = xs[0:128, None] * mean[None, 0:256] + mean[None, 256:512]
        cbv[:, 256:512] = xs[128:256, None] * mean[None, 0:256] + mean[None, 256:512]
        cbv[:, 512:768] = mean[None, 512:768]
        cbv[:, 768:800] = (L[768, 768] * z8 + mean[768]).reshape(NCT, 128).T

        in_maps.append({"zr": zr, "zx": zxr, "ltr": ltr,
                        "cb": cbv.astype(bf16)})
    return in_maps


def _assemble(results):
    out = np.empty((S, B), dtype=np.float32)
    for c in range(NCORES):
        o = results[c]["out"]                       # [128, 32]
        oc = o.reshape(128, S, 2).transpose(1, 2, 0).reshape(S, BC)
        out[:, c * BC:(c + 1) * BC] = oc
    return out


def _run(inputs, trace=False, trace_kwargs=None):
    from concourse.bass_utils import run_bass_kernel_spmd

    if "prog" not in _cache:
        _cache["prog"] = _build_program()
    nc = _cache["prog"]

    in_maps = _prep_inputs(**inputs)
    kw = {}
    if trace:
        kw["trace"] = True
        if trace_kwargs:
            kw.update(trace_kwargs)
    res = run_bass_kernel_spmd(nc, in_maps, core_ids=list(range(NCORES)), **kw)
    return _assemble(res.results), res


def kernel(x, mean, cov_vector, z):
    out, _ = _run(dict(x=np.asarray(x), mean=np.asarray(mean),
                       cov_vector=np.asarray(cov_vector), z=np.asarray(z)))
    return out
